# revision 1
# baseline (speedup 1.0000x reference)
"""ALSH Conv2d distributed Trainium2 kernel (8 NeuronCores).

Strategy:
  - Data-parallel over batch: 16 images -> 2 per core, one image per
    64-partition half of SBUF (image0 on partitions 0-63, image1 on 64-127).
  - Conv as 9 shifted K=64 matmuls per output chunk, with the two images'
    matmuls issued as concurrent row-tiled pairs (tile_position (0,0) and
    (64,0)) so the 128x128 PE array stays fully busy. bf16 matmul dtype
    (fp32 accumulation in PSUM).
  - The LSH vote v = a[:576] @ patch is factored through t[r, m] =
    sum_c a2[c, r] * xp[c, m] (one M=9 matmul per chunk), followed by
    shifted gather-DMAs + DVE adds, a magic-number floor bucketize, an
    8-bin histogram, an 8-core AllGather of the per-core histograms, and
    an on-device per-partition argmax -> one-hot -> factor-vector against
    a host-precomputed 128x8 factor table (the kernel-side hash depends
    only on the weights, so it is computed bit-exactly on host, jax/cpu).
  - A dummy 4-byte AllGather issued at kernel start absorbs the CC
    stream's once-per-NEFF entry barrier under the conv; 10 warm-up
    matmuls open the PE HAM clock gate before the conv begins.
  - Conv outputs are evicted unscaled to per-image SBUF staging (DVE/ACT
    split), scaled by the factor vector once the vote lands, and stored
    as contiguous padded-grid rows; the host strips the padding during
    the unshard (as it adds it during the shard).
"""

import numpy as np

OC, IC, KS, R_LSH, T_TBL = 128, 64, 3, 2.5, 8
SPAN = KS * KS * IC          # 576
B_FULL, H, W = 16, 56, 56
NCORES = 8
IMG_PER_CORE = B_FULL // NCORES   # 2
HP, WP = H + 2, W + 2        # 58x58 padded grid
GRID = HP * WP               # 3364
MARG = 64                    # zero margin columns on each side of the grid
LCOLS = MARG + GRID + MARG   # 3492
# chunk row counts over the 58 padded rows; all chunks >= 256 cols so fp32r
# runs at full rate (464,464,464,464,464,464,290,290)
CHUNK_ROWS = [8, 8, 8, 8, 8, 8, 5, 5]
ROW_START = [0, 8, 16, 24, 32, 40, 48, 53]
NCHUNK = len(CHUNK_ROWS)
MAXCHUNK = 8 * WP            # 464
SHIFTS = [((r // 3) - 1) * WP + ((r % 3) - 1) for r in range(9)]


def _build_graph(bias_u):
    """Build the 8-core Bass graph. bias_u = (0.5*sum(a[576:]) + b) / R."""
    import concourse.bass as bass
    import concourse.bacc as bacc
    import bass_rust
    import concourse.mybir as mybir
    from concourse import tile

    f32 = mybir.dt.float32
    bf16 = mybir.dt.bfloat16
    AX = mybir.AxisListType.X
    AF = mybir.ActivationFunctionType
    OP = mybir.AluOpType

    nc = bacc.Bacc("TRN2", target_bir_lowering=False, debug=False,
                   num_devices=NCORES)

    x_d = nc.dram_tensor("x", [IMG_PER_CORE, IC, LCOLS], bf16, kind="ExternalInput")
    wt_d = nc.dram_tensor("wt", [128, 9 * OC], bf16, kind="ExternalInput")
    a2_d = nc.dram_tensor("a2", [128, 9], bf16, kind="ExternalInput")
    ft_d = nc.dram_tensor("ftT", [OC, T_TBL], f32, kind="ExternalInput")
    io_d = nc.dram_tensor("iota8", [128, T_TBL], f32, kind="ExternalInput")
    out_d = nc.dram_tensor("out", [IMG_PER_CORE, OC, GRID], f32, kind="ExternalOutput")

    with tile.TileContext(nc) as tc:
        with (
            tc.tile_pool(name="const", bufs=1) as cpool,
            tc.tile_pool(name="xp", bufs=1) as xpool,
            tc.tile_pool(name="tg", bufs=1) as tgpool,
            tc.tile_pool(name="stg", bufs=1) as stgpool,
            tc.tile_pool(name="vv", bufs=9) as vvpool,
            tc.tile_pool(name="vsmall", bufs=1) as vspool,
            tc.tile_pool(name="tiny", bufs=1) as typool,
            tc.tile_pool(name="psc", bufs=6, space="PSUM") as psc,
            tc.tile_pool(name="pst", bufs=2, space="PSUM") as pst,
            tc.tile_pool(name="dram", bufs=2, space="DRAM") as dram,
        ):
            # ---- constants ----
            w_sb = cpool.tile([128, 9 * OC], bf16, tag="wsb")
            a2_sb = cpool.tile([128, 9], bf16, tag="a2sb")
            ft_sb = cpool.tile([OC, T_TBL], f32, tag="ftsb")
            iota_sb = cpool.tile([128, T_TBL], f32, tag="iosb")
            nc.sync.dma_start(ft_sb[:], ft_d[:])
            nc.sync.dma_start(iota_sb[:], io_d[:])

            # ---- padded input xp: [128, LCOLS]; partitions 0-63 image0,
            #      64-127 image1; grid position m lives at column MARG+m.
            # x arrives host-padded to the full [64, LCOLS] row layout
            # (zero margins + zero-padded 58x58 grid), so each image is one
            # casting DMA and every byte of xp is initialized. ----
            # dummy 4-byte AllGather issued immediately: the CC-stream's
            # once-per-NEFF entry barrier completes under the conv instead of
            # delaying the real histogram AllGather
            dcin = dram.tile([1, 1], f32, tag="dcin")
            dcout = dram.tile([NCORES, 1], f32, tag="dcout")
            nc.gpsimd.dma_start(dcin[:], iota_sb[0:1, 0:1])
            nc.gpsimd.collective_compute(
                "AllGather",
                mybir.AluOpType.bypass,
                replica_groups=[list(range(NCORES))],
                ins=[dcin[:].opt()],
                outs=[dcout[:].opt()],
            )
            xp = xpool.tile([128, LCOLS], bf16, tag="xp")
            nc.gpsimd.dma_start(xp[0:64, 0:LCOLS // 2], x_d[0][:, 0:LCOLS // 2])
            nc.gpsimd.dma_start(xp[0:64, LCOLS // 2:], x_d[0][:, LCOLS // 2:])
            nc.gpsimd.dma_start(xp[64:128, 0:LCOLS // 2], x_d[1][:, 0:LCOLS // 2])
            nc.gpsimd.dma_start(xp[64:128, LCOLS // 2:], x_d[1][:, LCOLS // 2:])

            nc.sync.dma_start(w_sb[:], wt_d[:])
            nc.sync.dma_start(a2_sb[:], a2_d[:])

            # ---- PE warm-up: ~4us of dense matmuls (on a memset tile, so
            # no load dependency) so the HAM clock gate opens (K=8/8,
            # 2.4 GHz) before the real conv begins ----
            wups = psc.tile([128, MAXCHUNK], f32, tag="psconv")
            for wi in range(10):
                nc.tensor.matmul(wups[:, 0:MAXCHUNK],
                                 w_sb[0:128, 0:128],
                                 w_sb[0:128, 0:MAXCHUNK],
                                 start=True, stop=True)

            # ================= vote stage 1: t[r, m] = sum_c a2[c,r]*xp[c,m] ==
            t_sbA = tgpool.tile([9, LCOLS], f32, tag="tsbA")
            t_sbB = tgpool.tile([9, LCOLS], f32, tag="tsbB")
            for k in range(NCHUNK):
                ncols = CHUNK_ROWS[k] * WP
                c0 = MARG + ROW_START[k] * WP
                tpsA = pst.tile([128, MAXCHUNK], f32, tag="tps")
                tpsB = pst.tile([128, MAXCHUNK], f32, tag="tps")
                nc.tensor.matmul(tpsA[0:9, 0:ncols],
                                 a2_sb[0:64, :],
                                 xp[0:64, c0:c0 + ncols],
                                 start=True, stop=True, tile_position=(0, 0))
                nc.tensor.matmul(tpsB[0:9, 0:ncols],
                                 a2_sb[64:128, :],
                                 xp[64:128, c0:c0 + ncols],
                                 start=True, stop=True, tile_position=(64, 0))
                nc.vector.tensor_copy(t_sbA[:, c0:c0 + ncols], tpsA[0:9, 0:ncols])
                nc.scalar.copy(t_sbB[:, c0:c0 + ncols], tpsB[0:9, 0:ncols])

            # ---- shifted gathers: vvr[y, x] = t[r, (y+1+dy)*58 + (x+1+dx)] ----
            # split across the two HWDGE queues (sync + scalar)
            vvr_tiles = []
            for r in range(9):
                vvr = vvpool.tile([112, W], f32, tag="vvr")
                off = MARG + SHIFTS[r] + WP + 1
                for i, tsb in enumerate((t_sbA, t_sbB)):
                    src = tsb[r:r + 1, off:off + H * WP] \
                        .rearrange("p (y x) -> p y x", x=WP)[:, :, 0:W]
                    eng = (nc.sync, nc.gpsimd, nc.sync)[(2 * r + i) % 3]
                    eng.dma_start(vvr[56 * i:56 * i + 56, :], src)
                vvr_tiles.append(vvr)

            # ================= main conv =================
            # per-image contiguous staging over the whole padded grid; chunk
            # evictions land at their grid offsets, then 3 fused scale ops +
            # 3 out-DMAs per image cover row groups [0,24), [24,48), [48,58)
            stg_img0 = stgpool.tile([128, GRID], f32, tag="stg0")
            stg_img1 = stgpool.tile([128, GRID], f32, tag="stg1")
            stg_imgs = [stg_img0, stg_img1]
            for k in range(NCHUNK):
                ncols = CHUNK_ROWS[k] * WP
                c0 = MARG + ROW_START[k] * WP
                g0 = ROW_START[k] * WP
                psA = psc.tile([128, MAXCHUNK], f32, tag="psconv")
                psB = psc.tile([128, MAXCHUNK], f32, tag="psconv")
                for r in range(9):
                    s = SHIFTS[r]
                    nc.tensor.matmul(psA[:, 0:ncols],
                                     w_sb[0:64, r * OC:(r + 1) * OC],
                                     xp[0:64, c0 + s:c0 + s + ncols],
                                     start=(r == 0), stop=(r == 8),
                                     tile_position=(0, 0))
                    nc.tensor.matmul(psB[:, 0:ncols],
                                     w_sb[64:128, r * OC:(r + 1) * OC],
                                     xp[64:128, c0 + s:c0 + s + ncols],
                                     start=(r == 0), stop=(r == 8),
                                     tile_position=(64, 0))
                nc.vector.tensor_copy(stg_imgs[0][:, g0:g0 + ncols], psA[:, 0:ncols])
                nc.scalar.copy(stg_imgs[1][:, g0:g0 + ncols], psB[:, 0:ncols])

            # ordering helper: force the vote mid-chain (DVE) to schedule
            # after the last conv eviction so evictions never stall the PE
            votegate = vspool.tile([1, 1], f32, tag="votegate")
            nc.vector.tensor_copy(votegate[:], stg_imgs[1][0:1, 0:1])

            # ---- v = sum_r vvr ; bucketize ; histogram ; vote ----
            # The whole chain runs on GpSimd (plus its SWDGE DMAs), so the
            # in-order DVE/ACT/PE streams serving the conv are never blocked
            # behind vote dependencies.
            # floor via the magic-number round-to-nearest trick (no floor/mod
            # ALU op): rni(z) = (z + MAGIC) - MAGIC for |z| < 2^22.
            MAGIC = 12582912.0  # 1.5 * 2^23
            g = nc.gpsimd
            acc = vspool.tile([112, W], f32, tag="acc")
            nc.vector.tensor_copy(acc[0:1, 0:1], votegate[:])
            nc.vector.tensor_tensor(acc[:], vvr_tiles[0][:], vvr_tiles[1][:], OP.add)
            for r in range(2, 9):
                nc.vector.tensor_tensor(acc[:], acc[:], vvr_tiles[r][:], OP.add)
            u_t = vspool.tile([112, W], f32, tag="ut")
            nc.scalar.activation(u_t[:], acc[:], AF.Copy, bias=float(bias_u), scale=float(1.0 / R_LSH))
            u2 = vspool.tile([112, W], f32, tag="u2")
            nc.vector.tensor_scalar(u2[:], u_t[:], 0.49995, MAGIC, OP.subtract, OP.add)
            q_t = vspool.tile([112, W], f32, tag="qt")
            nc.vector.tensor_scalar(q_t[:], u2[:], MAGIC, None, OP.subtract)
            aq = vspool.tile([112, W], f32, tag="aq")
            nc.scalar.activation(aq[:], q_t[:], AF.Abs)
            d1 = vspool.tile([112, W], f32, tag="d1")
            nc.vector.tensor_scalar(d1[:], aq[:], 0.125, 0.499, OP.mult, OP.subtract)
            d2 = vspool.tile([112, W], f32, tag="d2")
            nc.vector.tensor_scalar(d2[:], d1[:], MAGIC, MAGIC, OP.add, OP.subtract)
            votes = vspool.tile([112, W], f32, tag="votes")
            nc.vector.scalar_tensor_tensor(votes[:], d2[:], -8.0, aq[:], OP.mult, OP.add)

            hist = vspool.tile([112, T_TBL], f32, tag="hist")
            eq = vspool.tile([112, W], f32, tag="eq")
            for t in range(T_TBL):
                nc.vector.tensor_scalar(eq[:], votes[:], float(t), None, OP.is_equal)
                nc.vector.reduce_sum(hist[:, t:t + 1], eq[:], AX)
            hist_row = vspool.tile([1, T_TBL], f32, tag="histrow")
            g.reduce_sum(hist_row[0:1, :], hist[:], mybir.AxisListType.C)

            # ---- AllGather of per-core histograms ----
            cin = dram.tile([1, T_TBL], f32, tag="cin")
            cout = dram.tile([NCORES, T_TBL], f32, tag="cout")
            g.dma_start(cin[:], hist_row[0:1, :])
            g.collective_compute(
                "AllGather",
                mybir.AluOpType.bypass,
                replica_groups=[list(range(NCORES))],
                ins=[cin[:].opt()],
                outs=[cout[:].opt()],
            )
            # broadcast the gathered histograms to all 128 partitions in one
            # DMA, then run the whole argmax/factor chain per-partition on
            # the DVE with no cross-engine hops
            hs_bc = typool.tile([128, NCORES * T_TBL], f32, tag="hsbc")
            nc.sync.dma_start(hs_bc[:], cout[:].rearrange("r t -> (r t)")
                              .unsqueeze(0).broadcast_to([128, NCORES * T_TBL]))
            tot_bc = typool.tile([128, T_TBL], f32, tag="totbc")
            hs_v = hs_bc[:, :].rearrange("p (r t) -> p t r", t=T_TBL)
            nc.vector.reduce_sum(tot_bc[:], hs_v, AX)
            score = typool.tile([128, T_TBL], f32, tag="score")
            nc.vector.scalar_tensor_tensor(score[:], tot_bc[:], float(T_TBL), iota_sb[:],
                                           OP.mult, OP.subtract)
            mx = typool.tile([128, 1], f32, tag="mx")
            nc.vector.reduce_max(mx[:], score[:], AX)
            eqb = typool.tile([128, T_TBL], f32, tag="eqb")
            nc.vector.tensor_scalar(eqb[:], score[:], mx[:, 0:1], None, OP.is_equal)
            fvt = typool.tile([128, T_TBL], f32, tag="fvt")
            nc.vector.tensor_tensor(fvt[:], ft_sb[:], eqb[:], OP.mult)
            fv_sb = typool.tile([128, 1], f32, tag="fvsb")
            nc.vector.reduce_sum(fv_sb[:], fvt[:], AX)


            # ---- scale by factor vector, then DMA out valid rows ----
            # row groups (in padded rows): [0,24) [24,48) [48,58)
            GROUPS = [(0, 24), (24, 48), (48, 58)]
            ei = 0
            for gi, (r0, r1) in enumerate(GROUPS):
                for i in range(IMG_PER_CORE):
                    stg = stg_imgs[i]
                    ncols = (r1 - r0) * WP
                    if ei % 2 == 0:
                        fvb = fv_sb[:, 0:1].broadcast_to([128, ncols])
                        nc.vector.tensor_tensor(stg[:, r0 * WP:r1 * WP],
                                                stg[:, r0 * WP:r1 * WP], fvb, OP.mult)
                    else:
                        nc.scalar.activation(stg[:, r0 * WP:r1 * WP],
                                             stg[:, r0 * WP:r1 * WP],
                                             AF.Copy, scale=fv_sb[:, 0:1])
                    # contiguous padded-grid store; host strips the padding
                    oeng = nc.sync if ei % 2 == 0 else nc.scalar
                    oeng.dma_start(out_d[i, :, r0 * WP:r1 * WP],
                                   stg[:, r0 * WP:r1 * WP])
                    ei += 1

    nc.compile()
    return nc


def _host_prep(kernels, a, b):
    """Host-side weight layouts + bit-exact factor table via jax on CPU."""
    import jax
    import jax.numpy as jnp

    cpu = jax.devices("cpu")[0]
    k_j = jax.device_put(jnp.asarray(kernels, jnp.float32), cpu)
    a_j = jax.device_put(jnp.asarray(a, jnp.float32), cpu)
    b_j = jax.device_put(jnp.asarray(b, jnp.float32), cpu)

    norms2 = jnp.sum(k_j * k_j, axis=1)
    powers = jnp.stack([norms2 ** (2 ** i) for i in range(5)], axis=1)
    hk = k_j @ a_j[:SPAN] + powers @ a_j[SPAN:]
    kidx = np.asarray(jnp.abs(jnp.fmod(jnp.floor((hk + b_j) / R_LSH).astype(jnp.int32), T_TBL)))

    ftT = np.zeros((T_TBL, OC), np.float32)
    for t in range(T_TBL):
        mask = (kidx == t).astype(np.float32)
        cnt = mask.sum()
        if cnt > 0:
            ftT[t] = mask * np.float32(OC / max(cnt, np.float32(1.0)))
        else:
            ftT[t] = 1.0
    ftT = np.ascontiguousarray(ftT.T)  # [OC, T_TBL], oc on partitions

    c0 = 0.5 * float(jnp.sum(a_j[SPAN:]))
    bias_u = (c0 + float(b_j)) / R_LSH

    import ml_dtypes
    wt_half = np.asarray(kernels, np.float32).reshape(OC, IC, 9).transpose(1, 2, 0)  # [64, 9, 128]
    wt = np.concatenate([wt_half, wt_half], axis=0).reshape(128, 9 * OC)
    wt = np.ascontiguousarray(wt.astype(ml_dtypes.bfloat16))

    a2_half = np.asarray(a, np.float32)[:SPAN].reshape(IC, 9)
    a2 = np.ascontiguousarray(
        np.concatenate([a2_half, a2_half], axis=0).astype(ml_dtypes.bfloat16))

    iota8 = np.ascontiguousarray(np.tile(np.arange(T_TBL, dtype=np.float32), (128, 1)))
    return wt, a2, ftT, iota8, bias_u


def _pad_shard(xs):
    """[n, 64, 56, 56] -> bf16 [n, 64, LCOLS]: margins + padded 58x58 grid."""
    import ml_dtypes
    n = xs.shape[0]
    out = np.zeros((n, IC, LCOLS), ml_dtypes.bfloat16)
    grid = np.pad(xs, ((0, 0), (0, 0), (1, 1), (1, 1)))
    out[:, :, MARG:MARG + GRID] = grid.reshape(n, IC, GRID).astype(ml_dtypes.bfloat16)
    return np.ascontiguousarray(out)


def kernel(x, kernels, a, b, mode=0, **_ignored):
    from concourse.bass_utils import run_bass_kernel_spmd

    x = np.ascontiguousarray(np.asarray(x, np.float32))
    kernels = np.asarray(kernels, np.float32)
    a = np.asarray(a, np.float32)

    wt, a2, ftT, iota8, bias_u = _host_prep(kernels, a, b)
    nc = _build_graph(bias_u)

    in_maps = []
    for c in range(NCORES):
        in_maps.append({
            "x": _pad_shard(x[IMG_PER_CORE * c:IMG_PER_CORE * (c + 1)]),
            "wt": wt,
            "a2": a2,
            "ftT": ftT,
            "iota8": iota8,
        })
    res = run_bass_kernel_spmd(nc, in_maps, core_ids=list(range(NCORES)))
    out_pad = np.concatenate([res.results[c]["out"] for c in range(NCORES)], axis=0)
    out = np.ascontiguousarray(
        out_pad.reshape(B_FULL, OC, HP, WP)[:, :, 1:1 + H, 1:1 + W])
    return out

